# revision 1
# baseline (speedup 1.0000x reference)
"""AdaptiveSpectrumLayer Trainium2 kernel (8-core data-parallel).

Algorithm notes (validated against a numpy prototype):
  * rfft/irfft over the H=512 time axis are DFT matmuls with precomputed
    cos/sin matrices, packed so the 257 rfft bins fit 4 M-tiles of 128:
    [re 0..127][re 128..255][re256; im 1..127][im 128..255].
  * The per-frequency Linear(3->16) and the gate Linear(4112->257) fold
    into one tensor A[n,c,m] = sum_h W_proj[n,c,h] W_gate[m,16n+h], and
    bias_eff[m] = b_gate[m] + sum_{n,h} b_proj[n,h] W_gate[m,16n+h],
    so the whole gate path is one K=771 matmul (203M MACs, 0.9MB weights
    instead of 1.1G MACs / 4.2MB).  bias enters via an all-ones K-row.
  * softmax needs no max-subtraction (|logits| < ~20, fp32) and its
    normalization is deferred through the linear irfft: divide by the
    exp-sum only on the final output tiles.
  * The Nyquist bin's irfft contribution is folded into row 0 of the
    u_im K-tile (whose natural row, im[0]*e[0], is zero).
Sharded batch 128 -> 8 cores x 16.  All matmuls run as float32r
(full-rate 4-byte matmul mode for free-dim >= 256).
"""

import numpy as np

B, H, F = 128, 512, 64
HID = 16
NF = H // 2 + 1          # 257
NCORES = 8
BL = B // NCORES         # 16 batch per core
BF = BL * F              # 1024 free columns per core
P = 128
CH = 512                 # free-dim chunk (8 batches x 64)
NCH = BF // CH           # 2


def _build_constants(W_proj, b_proj, W_gate, b_gate):
    W_proj = np.asarray(W_proj, np.float64)
    b_proj = np.asarray(b_proj, np.float64)
    W_gate = np.asarray(W_gate, np.float64)
    b_gate = np.asarray(b_gate, np.float64)

    Wg = W_gate.reshape(NF, NF, HID)                      # [m, n, h]
    A = np.einsum("nch,mnh->ncm", W_proj, Wg)             # (257, 3, 257)
    bias_eff = b_gate + np.einsum("nh,mnh->m", b_proj, Wg)

    h = np.arange(H)
    n = np.arange(NF)
    ang = 2.0 * np.pi * np.outer(h, n) / H                # (512, 257)
    Cf = np.cos(ang)
    Sf = -np.sin(ang)

    # forward DFT weights: (512 h, 4*128 packed outputs); tile2 col0 is
    # zeroed (re256 gets its own M=1 matmul) so T2 row0 == im[0] == 0.
    Wf = np.concatenate(
        [
            Cf[:, 0:128],
            Cf[:, 128:256],
            np.concatenate([np.zeros((H, 1)), Sf[:, 1:128]], axis=1),
            Sf[:, 128:256],
        ],
        axis=1,
    ).astype(np.float32)                                  # (512, 512)
    Wf_nyq = np.ascontiguousarray(Cf[:, 256:257]).astype(np.float32)  # (512, 1)

    # gate weights: 8 K-tiles x 128 x 257
    Ap = np.zeros((8, P, NF), dtype=np.float32)
    Ap[0] = A[0:128, 0, :]
    Ap[1] = A[128:256, 0, :]
    Ap[2] = A[0:128, 1, :]
    Ap[2, 0] = bias_eff          # sin[0] is always 0 -> row reused for bias
    Ap[3] = A[128:256, 1, :]
    Ap[4] = A[0:128, 2, :]
    Ap[5] = A[128:256, 2, :]
    Ap[6, 0] = A[256, 0, :]      # mag256 row (K=1 tile)
    Ap[7, 0] = A[256, 2, :]      # cos256 row (K=1 tile)

    # inverse DFT weights: 4 K-tiles x 128 x 512
    cinv = np.cos(ang)
    sinv = np.sin(ang)
    cn = np.full(NF, 2.0)
    cn[0] = 1.0
    cn[256] = 1.0
    Ci = cinv * cn[None, :] / H                           # (512, 257)
    Si = (-2.0 / H) * sinv
    Wi = np.zeros((4, P, H), dtype=np.float32)
    Wi[0] = Ci[:, 0:128].T
    Wi[1] = Ci[:, 128:256].T
    Wi[2, 0] = Ci[:, 256]                                 # Nyquist row
    Wi[2, 1:128] = Si[:, 1:128].T
    Wi[3] = Si[:, 128:256].T

    # SBUF-layout packs (partition-major, fully contiguous DMAs):
    import ml_dtypes  # noqa
    Wf_p = np.ascontiguousarray(
        np.concatenate([Wf.reshape(4, P, 512).transpose(1, 0, 2),
                        Wf_nyq.reshape(4, P, 1).transpose(1, 0, 2)],
                       axis=2)).astype(ml_dtypes.bfloat16)  # (P, 4k, 513) bf16
    Ap_p = np.ascontiguousarray(Ap.transpose(1, 0, 2))     # (P, 8k, 257)
    Wi_p = np.ascontiguousarray(
        Wi.transpose(1, 0, 2)).astype(ml_dtypes.bfloat16)  # (P, 4k, 512) bf16
    return Wf_p, Ap_p, Wi_p


def _build_graph(reps=1):
    from contextlib import ExitStack

    import concourse.bass as bass
    import concourse.tile as tile
    from concourse import bacc, mybir

    F32 = mybir.dt.float32
    FR = mybir.dt.float32r
    BF16 = mybir.dt.bfloat16
    AF = mybir.ActivationFunctionType

    nc = bacc.Bacc(
        "TRN2",
        target_bir_lowering=False,
        debug=False,
        num_devices=NCORES,
    )

    # all DRAM layouts are partition-major so every DMA is fully contiguous
    x_ext = nc.dram_tensor("x", [4, P, BL, F], BF16, kind="ExternalInput").ap()
    wf_ext = nc.dram_tensor("wf", [P, 4, 513], BF16, kind="ExternalInput").ap()
    ap_ext = nc.dram_tensor("apk", [P, 8, NF], FR, kind="ExternalInput").ap()
    wi_ext = nc.dram_tensor("wi", [P, 4, H], BF16, kind="ExternalInput").ap()
    out_ext = nc.dram_tensor("out", [4, P, BL, F], F32, kind="ExternalOutput").ap()

    with tile.TileContext(nc) as tc, ExitStack() as ctx:
        const = ctx.enter_context(tc.tile_pool(name="const", bufs=1))
        tpool = ctx.enter_context(tc.tile_pool(name="reim", bufs=1))
        fpool = ctx.enter_context(tc.tile_pool(name="feats", bufs=1))
        spool = ctx.enter_context(tc.tile_pool(name="small", bufs=1))
        wpool = ctx.enter_context(tc.tile_pool(name="work", bufs=1))
        opool = ctx.enter_context(tc.tile_pool(name="outs", bufs=1))
        psmm = ctx.enter_context(tc.tile_pool(name="psmm", bufs=3, space="PSUM"))
        psy = ctx.enter_context(tc.tile_pool(name="psy", bufs=3, space="PSUM"))
        pssm = ctx.enter_context(tc.tile_pool(name="pssm", bufs=2, space="PSUM"))

        # ---- constants / input DMA (all contiguous)
        wf_sb = const.tile([P, 4, 513], BF16, tag="wf", name="wf")
        for k in range(4):
            [nc.sync, nc.gpsimd][k % 2].dma_start(wf_sb[:, k, :], wf_ext[:, k, :])
        ap_sb = const.tile([P, 8, NF], FR, tag="apk", name="apk")
        wi_sb = const.tile([P, 4, H], BF16, tag="wi", name="wi")
        ones_f32 = const.tile([P, 1], F32, tag="ones_f32", name="ones_f32")
        nc.vector.memset(ones_f32[:], 1.0)
        onesr_f32 = const.tile([1, P + BF], F32, tag="onesr_f32", name="onesr_f32")
        nc.vector.memset(onesr_f32[:], 1.0)
        ones_bf = const.tile([P, 1], BF16, tag="ones_bf", name="ones_bf")
        nc.vector.memset(ones_bf[:], 1.0)
        ones_row = const.tile([1, P], FR, tag="ones_row", name="ones_row")
        nc.vector.tensor_copy(ones_row[:], onesr_f32[0:1, 0:P])
        warm = const.tile([1, 8], F32, tag="warm", name="warm")
        nc.scalar.activation(warm[:], onesr_f32[0:1, 0:8], func=AF.Sqrt)

        for _rep in range(reps):
            x_sb = const.tile([P, 4, BL, F], BF16, tag="x", name="x")
            bs0 = slice(0, CH // F)
            for k in range(4):
                [nc.sync, nc.gpsimd][k % 2].dma_start(x_sb[:, k, bs0, :], x_ext[k][:, bs0, :])
            bs1 = slice(CH // F, 2 * (CH // F))
            for k in range(4):
                [nc.sync, nc.gpsimd][k % 2].dma_start(x_sb[:, k, bs1, :], x_ext[k][:, bs1, :])
            if _rep == 0:
                nc.sync.dma_start(ap_sb[:], ap_ext)
                nc.gpsimd.dma_start(wi_sb[:], wi_ext)

            # T layout: j = [re_a, im_a, re_b, im_b]; fwd m-tile -> j map
            Tc = [tpool.tile([P, 4, CH], F32, tag=f"tc{c}", name=f"tc{c}")
                  for c in range(NCH)]
            perm = [0, 2, 1, 3]
            mags, scs, nyq, r256s = {}, {}, {}, {}

            # ============ phase A: forward DFT + feats (sqrt set) =========
            for c in range(NCH):
                bsl = slice(c * (CH // F), (c + 1) * (CH // F))
                for mt in range(4):
                    ps = psmm.tile([P, CH], F32, tag="mm", name="mm")
                    for k in range(4):
                        nc.tensor.matmul(
                            ps[:],
                            wf_sb[:, k, mt * P:(mt + 1) * P],
                            x_sb[:, k, bsl, :],
                            start=(k == 0),
                            stop=(k == 3),
                        )
                    if mt < 2:
                        nc.vector.tensor_copy(Tc[c][:, perm[mt], :], ps[:])
                    else:
                        nc.scalar.activation(Tc[c][:, perm[mt], :], ps[:], func=AF.Copy)

                psn = pssm.tile([1, CH], F32, tag="sm", name="psn")
                for k in range(4):
                    nc.tensor.matmul(
                        psn[:],
                        wf_sb[:, k, 512:513],
                        x_sb[:, k, bsl, :],
                        start=(k == 0),
                        stop=(k == 3),
                    )
                r256 = spool.tile([1, CH], F32, tag=f"r256_{c}", name=f"r256_{c}")
                nc.scalar.activation(r256[:], psn[:], func=AF.Copy)
                r256s[c] = r256

                # packed feats: square/add/sqrt/recip/sin-cos as wide ops;
                # chunk 0's chain is the critical path to the first y-matmul
                from contextlib import nullcontext
                with (tc.high_priority() if False else nullcontext()):
                    ssq = wpool.tile([P, 4, CH], F32, tag="ssq", name="ssq", bufs=2)
                    nc.scalar.activation(ssq[:], Tc[c][:, :, :], func=AF.Square)
                    sq = wpool.tile([P, 2, CH], F32, tag="sq", name="sq", bufs=2)
                    nc.vector.tensor_add(sq[:], ssq[:, 0:4:2, :], ssq[:, 1:4:2, :])
                    mg32 = wpool.tile([P, 2, CH], F32, tag="mg32", name="mg32", bufs=2)
                    nc.scalar.activation(mg32[:], sq[:], func=AF.Sqrt)
                    rinv = wpool.tile([P, 2, CH], F32, tag="rinv", name="rinv", bufs=2)
                    nc.vector.reciprocal(rinv[:], mg32[:])
                    mag = fpool.tile([P, 2, CH], FR, tag=f"mag_{c}", name=f"mag_{c}")
                    nc.gpsimd.tensor_copy(mag[:], mg32[:])
                    sc = fpool.tile([P, 4, CH], FR, tag=f"sc_{c}", name=f"sc_{c}")
                    nc.vector.tensor_mul(
                        sc[:],
                        Tc[c][:, :, :],
                        rinv[:, :, None, :].broadcast_to([P, 2, 2, CH]),
                    )
                mags[c], scs[c] = mag, sc
                # bias rides the always-zero sin[0] feature row (sc j=1 row0)
                nc.gpsimd.tensor_copy(sc[0:1, 1, :], onesr_f32[0:1, P:P + CH])
                # Nyquist K=1 rows
                ab256 = spool.tile([1, CH], F32, tag=f"ab256_{c}", name=f"ab256_{c}")
                nc.scalar.activation(ab256[:], r256[:], func=AF.Abs)
                ri256 = spool.tile([1, CH], F32, tag=f"ri256_{c}", name=f"ri256_{c}")
                nc.vector.reciprocal(ri256[:], ab256[:])
                mg256 = spool.tile([1, CH], FR, tag=f"mg256_{c}", name=f"mg256_{c}")
                nc.gpsimd.tensor_copy(mg256[:], ab256[:])
                cs256 = spool.tile([1, CH], FR, tag=f"cs256_{c}", name=f"cs256_{c}")
                nc.vector.tensor_mul(cs256[:], r256[:], ri256[:])
                nyq[c] = (mg256, cs256)

            # == phase B/C per chunk: logits, weights, u, inverse (exp set)
            zout = [
                opool.tile([P, NCH, CH], F32, tag=f"z{m}", name=f"z{m}")
                for m in range(4)
            ]
            for c in range(NCH):
                klist = [
                    (nyq[c][0][:], 6, 1),
                    (nyq[c][1][:], 7, 1),
                    (mags[c][:, 0, :], 0, P),
                    (mags[c][:, 1, :], 1, P),
                    (scs[c][:, 1, :], 2, P),
                    (scs[c][:, 3, :], 3, P),
                    (scs[c][:, 0, :], 4, P),
                    (scs[c][:, 2, :], 5, P),
                ]
                ey = []
                for mt in range(3):
                    msl = slice(mt * P, NF if mt == 2 else (mt + 1) * P)
                    mp = 1 if mt == 2 else P
                    pool = pssm if mt == 2 else psy
                    ps = pool.tile([mp, CH], F32, tag="sm" if mt == 2 else "y",
                                   name="psy")
                    for i, (rhs, kt, kk) in enumerate(klist):
                        nc.tensor.matmul(
                            ps[:],
                            ap_sb[0:kk, kt, msl],
                            rhs[0:kk],
                            start=(i == 0),
                            stop=(i == len(klist) - 1),
                        )
                    # silu(y) via tanh: e = exp(0.5 * (1 + tanh(y/2)) * y)
                    th = wpool.tile([mp, CH], F32, tag=f"th{mt}", name=f"th{mt}", bufs=3)
                    nc.scalar.activation(th[:], ps[:], func=AF.Tanh, scale=0.5)
                    ysw2 = wpool.tile([mp, CH], F32, tag=f"ysw{mt}", name=f"ysw{mt}", bufs=3)
                    nc.vector.scalar_tensor_tensor(
                        out=ysw2[:], in0=th[:], scalar=1.0, in1=ps[:],
                        op0=mybir.AluOpType.add, op1=mybir.AluOpType.mult,
                    )
                    e = fpool.tile([mp, CH], BF16, tag=f"e{mt}_{c}", name=f"e{mt}_{c}")
                    nc.scalar.activation(e[:], ysw2[:], func=AF.Exp, scale=0.5)
                    ey.append(e)

                # exp-sum (bf16 matmul) and reciprocal broadcast
                ps_s = pssm.tile([1, CH], F32, tag="sm", name="ps_s")
                nc.tensor.matmul(ps_s[:], ones_bf[:], ey[0][:], start=True, stop=False)
                nc.tensor.matmul(ps_s[:], ones_bf[:], ey[1][:], start=False, stop=False)
                nc.tensor.matmul(ps_s[:], ones_bf[0:1, :], ey[2][:], start=False, stop=True)
                srec = spool.tile([1, CH], FR, tag=f"srec_{c}", name=f"srec_{c}")
                with nc.allow_low_precision(reason="f32r softmax scale"):
                    nc.vector.reciprocal(srec[:], ps_s[:])
                ps_rb = pssm.tile([P, CH], F32, tag="sm", name="ps_rb")
                nc.tensor.matmul(ps_rb[:], ones_row[:], srec[:], start=True, stop=True)
                rb = fpool.tile([P, CH], F32, tag=f"rb_{c}", name=f"rb_{c}")
                nc.scalar.activation(rb[:], ps_rb[:], func=AF.Copy)

                # u = fft * e (unnormalized, bf16); /S deferred to epilogue
                Ua = fpool.tile([P, 2, CH], BF16, tag=f"ua_{c}", name=f"ua_{c}")
                nc.vector.tensor_mul(
                    Ua[:], Tc[c][:, 0:2, :],
                    ey[0][:, None, :].broadcast_to([P, 2, CH]),
                )
                nc.vector.tensor_mul(Ua[0:1, 1, :], r256s[c][:], ey[2][:])
                Ub = fpool.tile([P, 2, CH], BF16, tag=f"ub_{c}", name=f"ub_{c}")
                (nc.vector if c == 0 else nc.gpsimd).tensor_mul(
                    Ub[:], Tc[c][:, 2:4, :],
                    ey[1][:, None, :].broadcast_to([P, 2, CH]),
                )
                U = [Ua[:, 0, :], Ub[:, 0, :], Ua[:, 1, :], Ub[:, 1, :]]

                # inverse DFT + epilogue + output DMA for this chunk
                korder = [0, 2, 1, 3]  # Ua parts first; gpsimd-made Ub last
                for mt in range(4):
                    ps = psmm.tile([P, CH], F32, tag="mm", name="psz")
                    for j, k in enumerate(korder):
                        nc.tensor.matmul(
                            ps[:],
                            wi_sb[:, k, mt * P:(mt + 1) * P],
                            U[k],
                            start=(j == 0),
                            stop=(j == 3),
                        )
                    nc.vector.tensor_mul(zout[mt][:, c, :], ps[:], rb[:])
                    eng = [nc.sync, nc.gpsimd, nc.sync, nc.gpsimd][mt]
                    eng.dma_start(
                        out_ext[mt][:, c * (CH // F):(c + 1) * (CH // F), :],
                        zout[mt][:, c, :].rearrange("p (b f) -> p b f", f=F),
                    )

    nc.compile()
    return nc


_CACHE = {}


def _bf16():
    import ml_dtypes
    return ml_dtypes.bfloat16


def _pack_in_maps(inputs):
    # constants are cheap (~100ms numpy) relative to any device call;
    # rebuild every time so repeated kernel() calls with new weights are safe
    Wf, Ap, Wi = _build_constants(
        inputs["W_proj"], inputs["b_proj"], inputs["W_gate"], inputs["b_gate"]
    )
    x = np.ascontiguousarray(np.asarray(inputs["x"], np.float32))
    return [
        {
            # (BL,H,F) -> (4,P,BL,F): h-tile-major, partition-contiguous
            "x": np.ascontiguousarray(
                x[c * BL:(c + 1) * BL].transpose(1, 0, 2).reshape(4, P, BL, F)
            ).astype(_bf16()),
            "wf": Wf,
            "apk": Ap,
            "wi": Wi,
        }
        for c in range(NCORES)
    ]


def _run(inputs, trace=False):
    from concourse.bass_utils import run_bass_kernel_spmd

    if "graph" not in _CACHE:
        _CACHE["graph"] = _build_graph()
    nc = _CACHE["graph"]
    in_maps = _pack_in_maps(inputs)
    res = run_bass_kernel_spmd(nc, in_maps, core_ids=list(range(NCORES)), trace=trace)
    # (4,P,BL,F) -> (BL,H,F)
    out = np.concatenate(
        [r["out"].transpose(2, 0, 1, 3).reshape(BL, H, F) for r in res.results],
        axis=0,
    )
    return out.astype(np.float32), res


def kernel(**inputs):
    out, _ = _run(inputs, trace=False)
    return out


def _make_exec(nc):
    """Build a jit-cached 8-core executor for a compiled Bacc graph,
    replicating bass2jax.run_bass_via_pjrt's multi-core path but reusable
    across calls (for timing)."""
    import jax
    import numpy as np
    from jax.sharding import Mesh, PartitionSpec
    from jax.experimental.shard_map import shard_map
    from concourse import mybir
    from concourse.bass2jax import _bass_exec_p, install_neuronx_cc_hook

    install_neuronx_cc_hook()
    from concourse.bass2jax import partition_id_tensor

    n_cores = NCORES
    pid_name = nc.partition_id_tensor.name if nc.partition_id_tensor else None
    in_names, out_names, out_avals, zero_outs = [], [], [], []
    for alloc in nc.m.functions[0].allocations:
        if not isinstance(alloc, mybir.MemoryLocationSet):
            continue
        name = alloc.memorylocations[0].name
        if alloc.kind == "ExternalInput":
            if name != pid_name:
                in_names.append(name)
        elif alloc.kind == "ExternalOutput":
            out_names.append(name)
            shape = tuple(alloc.tensor_shape)
            dtype = mybir.dt.np(alloc.dtype)
            out_avals.append(jax.core.ShapedArray(shape, dtype))
            zero_outs.append(np.zeros(shape, dtype))
    n_params = len(in_names)
    all_names = in_names + out_names
    if pid_name is not None:
        all_names = all_names + [pid_name]

    def _body(*args):
        operands = list(args)
        if pid_name is not None:
            operands.append(partition_id_tensor())
        outs = _bass_exec_p.bind(
            *operands,
            out_avals=tuple(out_avals),
            in_names=tuple(all_names),
            out_names=tuple(out_names),
            lowering_input_output_aliases=(),
            sim_require_finite=True,
            sim_require_nnan=True,
            nc=nc,
        )
        return tuple(outs)

    devices = jax.devices()[:n_cores]
    mesh = Mesh(np.asarray(devices), ("core",))
    n_all = n_params + len(out_names)
    fn = jax.jit(
        shard_map(
            _body,
            mesh=mesh,
            in_specs=(PartitionSpec("core"),) * n_all,
            out_specs=(PartitionSpec("core"),) * len(out_names),
            check_rep=False,
        ),
        keep_unused=True,
    )

    def pack(in_maps):
        concat = [
            np.concatenate([np.asarray(in_maps[c][k]) for c in range(n_cores)], axis=0)
            for k in in_names
        ]
        concat += [
            np.zeros((n_cores * z.shape[0], *z.shape[1:]), z.dtype) for z in zero_outs
        ]
        return [jax.device_put(a) for a in concat]

    return fn, pack, out_names, out_avals



# revision 19
# speedup vs baseline: 1.6854x; 1.6854x over previous
"""AdaptiveSpectrumLayer Trainium2 kernel (8-core data-parallel), v2.

Structure (vs v1):
  * rfft/irfft over H=512 are DFT matmuls (bf16).  The 257 rfft bins pack
    4 M-tiles of 128: [re 0..127][re 128..255][re256 @row0; im 1..127]
    [im 128..255] — re256 rides the structurally-zero im[0] row, so the
    separate M=1 Nyquist matmul disappears (a 1-row fixup restores
    mag[0] = |re[0]| for the gate features).
  * Gate y = feats @ A is computed TRANSPOSED: out[col, m=257] with the
    feature axis contracted on partitions (7 K-tiles x 257-free matmuls),
    so the m=256 row no longer costs a full M=1 pass and softmax over m
    becomes a free-axis op: exp with accum_out gives the row sum for
    free, reciprocal_approx_fast + a per-partition scalar multiply
    normalizes w in-place.  Normalized w is transposed back to [n, col]
    with cheap 128-wide PE transposes; deferred-normalization epilogue
    from v1 is gone (plain PSUM->SBUF copies).
  * nc.vector.reciprocal (~6 cycles/elem on HW) is replaced everywhere
    by RECIPROCAL_APPROX_FAST custom-DVE ops (~51 ULP, 1 pass).
  * Elementwise work is spread across DVE / Act / Pool by measured cost;
    fft tiles (Tc) are bf16 so the u = fft*w multiplies hit the DVE
    2x 16-bit mode.
Sharded batch 128 -> 8 cores x 16.  Gate matmuls in float32r (free dim
257 >= 256 keeps full rate), DFT matmuls bf16.
"""

import numpy as np

B, H, F = 128, 512, 64
HID = 16
NF = H // 2 + 1          # 257
NCORES = 8
BL = B // NCORES         # 16 batch per core
BF = BL * F              # 1024 free columns per core
P = 128
CH = 512                 # free-dim chunk (8 batches x 64)
NCH = BF // CH           # 2
NQ = CH // P             # 4 col-subtiles of 128 per chunk


def _build_constants(W_proj, b_proj, W_gate, b_gate):
    W_proj = np.asarray(W_proj, np.float64)
    b_proj = np.asarray(b_proj, np.float64)
    W_gate = np.asarray(W_gate, np.float64)
    b_gate = np.asarray(b_gate, np.float64)

    Wg = W_gate.reshape(NF, NF, HID)                      # [m, n, h]
    A = np.einsum("nch,mnh->ncm", W_proj, Wg)             # (257, 3, 257)
    bias_eff = b_gate + np.einsum("nh,mnh->m", b_proj, Wg)

    h = np.arange(H)
    n = np.arange(NF)
    ang = 2.0 * np.pi * np.outer(h, n) / H                # (512, 257)
    Cf = np.cos(ang)
    Sf = -np.sin(ang)

    # forward DFT weights: (512 h, 4*128 packed outputs); tile2 col0 now
    # carries re256 (the im[0] row is structurally zero).
    Wf = np.concatenate(
        [
            Cf[:, 0:128],
            Cf[:, 128:256],
            np.concatenate([Cf[:, 256:257], Sf[:, 1:128]], axis=1),
            Sf[:, 128:256],
        ],
        axis=1,
    ).astype(np.float32)                                  # (512, 512)

    # gate rhs tiles (transposed gate): 8 K-tiles x 128 x 257
    # (tiles 6/7 are single-row: the Nyquist mag/cos features)
    Ap = np.zeros((8, P, NF), dtype=np.float32)
    Ap[0] = A[0:128, 0, :]
    Ap[1] = A[128:256, 0, :]
    Ap[2] = A[0:128, 1, :]
    Ap[2, 0] = bias_eff          # sin[0] is always 0 -> row reused for bias
    Ap[3] = A[128:256, 1, :]
    Ap[4] = A[0:128, 2, :]
    Ap[5] = A[128:256, 2, :]
    Ap[6, 0] = A[256, 0, :]      # mag256 row
    Ap[7, 0] = A[256, 2, :]      # cos256 row

    # inverse DFT weights: 4 K-tiles x 128 x 512
    cinv = np.cos(ang)
    sinv = np.sin(ang)
    cn = np.full(NF, 2.0)
    cn[0] = 1.0
    cn[256] = 1.0
    Ci = cinv * cn[None, :] / H                           # (512, 257)
    Si = (-2.0 / H) * sinv
    Wi = np.zeros((4, P, H), dtype=np.float32)
    Wi[0] = Ci[:, 0:128].T
    Wi[1] = Ci[:, 128:256].T
    Wi[2, 0] = Ci[:, 256]                                 # Nyquist row
    Wi[2, 1:128] = Si[:, 1:128].T
    Wi[3] = Si[:, 128:256].T

    import ml_dtypes  # noqa
    Wf_p = np.ascontiguousarray(
        Wf.reshape(4, P, 512).transpose(1, 0, 2)).astype(ml_dtypes.bfloat16)
    Ap_p = np.ascontiguousarray(
        Ap.transpose(1, 0, 2)).astype(ml_dtypes.bfloat16)  # (P, 8, 257) bf16
    Wi_p = np.ascontiguousarray(
        Wi.transpose(1, 0, 2)).astype(ml_dtypes.bfloat16)  # (P, 4, 512) bf16
    eye = np.eye(P, dtype=ml_dtypes.bfloat16)              # (P, 128)
    return Wf_p, Ap_p, Wi_p, eye


def _build_graph(reps=1):
    from contextlib import ExitStack

    import concourse.bass as bass
    import concourse.tile as tile
    from concourse import bacc, mybir
    from concourse.dve_ops import RECIP_APPROX_FAST_CONSTS, RECIPROCAL_APPROX_FAST

    F32 = mybir.dt.float32
    FR = mybir.dt.float32r
    BF16 = mybir.dt.bfloat16
    AF = mybir.ActivationFunctionType
    RC = RECIP_APPROX_FAST_CONSTS

    nc = bacc.Bacc(
        "TRN2",
        target_bir_lowering=False,
        debug=False,
        num_devices=NCORES,
    )

    def recip_fast(out_ap, in_ap):
        # raw emit: skips the f32-dtype assert so float32r tiles (same bit
        # layout) are accepted
        nc.vector._custom_dve(
            RECIPROCAL_APPROX_FAST, out=out_ap, in0=in_ap,
            s0=RC["s0"], s1=RC["s1"], imm2=RC["imm2"],
        )

    # all DRAM layouts are partition-major so every DMA is fully contiguous
    x_ext = nc.dram_tensor("x", [4, P, BL, F], BF16, kind="ExternalInput").ap()
    wf_ext = nc.dram_tensor("wf", [P, 4, 512], BF16, kind="ExternalInput").ap()
    ap_ext = nc.dram_tensor("apk", [P, 8, NF], BF16, kind="ExternalInput").ap()
    wi_ext = nc.dram_tensor("wi", [P, 4, H], BF16, kind="ExternalInput").ap()
    eye_ext = nc.dram_tensor("eye", [P, P], BF16, kind="ExternalInput").ap()
    out_ext = nc.dram_tensor("out", [4, P, BL, F], F32, kind="ExternalOutput").ap()

    with tile.TileContext(nc) as tc, ExitStack() as ctx:
        const = ctx.enter_context(tc.tile_pool(name="const", bufs=1))
        tpool = ctx.enter_context(tc.tile_pool(name="reim", bufs=1))
        fpool = ctx.enter_context(tc.tile_pool(name="feats", bufs=1))
        spool = ctx.enter_context(tc.tile_pool(name="small", bufs=1))
        wpool = ctx.enter_context(tc.tile_pool(name="work", bufs=1))
        opool = ctx.enter_context(tc.tile_pool(name="outs", bufs=1))
        psmm = ctx.enter_context(tc.tile_pool(name="psmm", bufs=2, space="PSUM"))
        psy = ctx.enter_context(tc.tile_pool(name="psy", bufs=3, space="PSUM"))
        pswt = ctx.enter_context(tc.tile_pool(name="pswt", bufs=2, space="PSUM"))
        pswn = ctx.enter_context(tc.tile_pool(name="pswn", bufs=1, space="PSUM"))

        # ---- constants / input DMA (all contiguous)
        wf_sb = const.tile([P, 4, 512], BF16, tag="wf", name="wf")
        for k in range(4):
            [nc.sync, nc.gpsimd][k % 2].dma_start(wf_sb[:, k, :], wf_ext[:, k, :])
        ap_sb = const.tile([P, 8, NF], BF16, tag="apk", name="apk")
        wi_sb = const.tile([P, 4, H], BF16, tag="wi", name="wi")
        eye_sb = const.tile([P, P], BF16, tag="eye", name="eye")
        onesr_f32 = const.tile([1, P + BF], F32, tag="onesr_f32", name="onesr_f32")
        nc.vector.memset(onesr_f32[:], 1.0)
        warm = const.tile([1, 8], F32, tag="warm", name="warm")
        nc.scalar.activation(warm[:], onesr_f32[0:1, 0:8], func=AF.Sqrt)

        xpool = ctx.enter_context(tc.tile_pool(name="xin", bufs=2))

        for _rep in range(reps):
            # input DMA rides the SP queue only; output DMAs use gpsimd's,
            # so next-rep input never queues behind this rep's output
            x_sb = xpool.tile([P, 4, BL, F], BF16, tag="x", name="x")
            for half in range(2):
                bs = slice(half * (CH // F), (half + 1) * (CH // F))
                for k in range(4):
                    nc.sync.dma_start(x_sb[:, k, bs, :], x_ext[k][:, bs, :])
            if _rep == 0:
                nc.sync.dma_start(ap_sb[:], ap_ext)
                nc.gpsimd.dma_start(wi_sb[:], wi_ext)
                nc.gpsimd.dma_start(eye_sb[:], eye_ext)

            # T layout: j = [re_a, im_a(re256@row0), re_b, im_b]
            Tc = [tpool.tile([P, 4, CH], BF16, tag=f"tc{c}", name=f"tc{c}")
                  for c in range(NCH)]
            perm = [0, 2, 1, 3]
            zout = [
                opool.tile([P, NCH, CH], F32, tag=f"z{m}", name=f"z{m}")
                for m in range(4)
            ]
            mags, scs, mg256s, cs256s, wTs, wnTs = {}, {}, {}, {}, {}, {}

            # ===== stage 1: forward DFTs (both chunks back-to-back on PE)
            # m-tile order (0,2,1,3) finishes group a (re_a, im_a) first so
            # its feature chain starts mid-fwd
            for c in range(NCH):
                bsl = slice(c * (CH // F), (c + 1) * (CH // F))
                for mt in (0, 2, 1, 3):
                    ps = psmm.tile([P, CH], F32, tag="mm", name="mm")
                    for k in range(4):
                        nc.tensor.matmul(
                            ps[:],
                            wf_sb[:, k, mt * P:(mt + 1) * P],
                            x_sb[:, k, bsl, :],
                            start=(k == 0),
                            stop=(k == 3),
                        )
                    if mt < 2:
                        nc.vector.tensor_copy(Tc[c][:, perm[mt], :], ps[:])
                    else:
                        nc.scalar.activation(Tc[c][:, perm[mt], :], ps[:], func=AF.Copy)

            # ===== stage 2: feats, group a then group b per chunk (short
            # serial chains so the gate can start right after fwd)
            for c in range(NCH):
                mag = fpool.tile([P, 2, CH], BF16, tag=f"mag_{c}", name=f"mag_{c}")
                sc = fpool.tile([P, 4, CH], BF16, tag=f"sc_{c}", name=f"sc_{c}")
                mg256 = spool.tile([1, CH], BF16, tag=f"mg256_{c}", name=f"mg256_{c}")
                ri256 = spool.tile([1, CH], BF16, tag=f"ri256_{c}", name=f"ri256_{c}")
                cs256 = spool.tile([1, CH], BF16, tag=f"cs256_{c}", name=f"cs256_{c}")
                ssq = wpool.tile([P, 4, CH], BF16, tag=f"ssq{c}", name=f"ssq{c}")
                sq = wpool.tile([P, 2, CH], BF16, tag=f"sq{c}", name=f"sq{c}")
                rinv = wpool.tile([P, 2, CH], BF16, tag=f"rinv{c}", name=f"rinv{c}")
                # nyquist chain first (long cross-engine path; im_a is ready
                # right after the second fwd m-tile)
                nc.scalar.activation(mg256[:], Tc[c][0:1, 1, :], func=AF.Abs)
                with nc.allow_low_precision(reason="bf16 features"):
                    recip_fast(ri256[:], mg256[:])
                    nc.gpsimd.tensor_mul(cs256[:], Tc[c][0:1, 1, :], ri256[:])
                for g in range(2):
                    js = slice(2 * g, 2 * g + 2)
                    with nc.allow_low_precision(reason="bf16 features"):
                        nc.vector.tensor_mul(ssq[:, js, :], Tc[c][:, js, :],
                                             Tc[c][:, js, :])
                        nc.vector.tensor_add(sq[:, g, :], ssq[:, 2 * g, :],
                                             ssq[:, 2 * g + 1, :])
                        if g == 0:
                            # row0: mag[0] = |re0| (im0 slot carries re256)
                            nc.vector.tensor_mul(sq[0:1, 0, :], Tc[c][0:1, 0, :],
                                                 Tc[c][0:1, 0, :])
                        nc.scalar.activation(mag[:, g, :], sq[:, g, :], func=AF.Sqrt)
                        recip_fast(rinv[:, g, :], mag[:, g, :])
                        nc.vector.tensor_mul(
                            sc[:, js, :],
                            Tc[c][:, js, :],
                            rinv[:, g, None, :].broadcast_to([P, 2, CH]),
                        )
                # bias rides the always-zero sin[0] feature row (sc j=1 row0)
                nc.gpsimd.tensor_copy(sc[0:1, 1, :], onesr_f32[0:1, P:P + CH])
                mags[c], scs[c], mg256s[c], cs256s[c] = mag, sc, mg256, cs256

            # ===== stage 3: gate + softmax for both chunks (PE dense)
            wqs = {}
            for c in range(NCH):
                mag, sc = mags[c], scs[c]
                mg256, cs256 = mg256s[c], cs256s[c]
                wq = []
                for q in range(NQ):
                    qsl = slice(q * P, (q + 1) * P)
                    # group-a features first: the gate can start before the
                    # group-b feature chain finishes
                    klist = [
                        (mag[:, 0, qsl], 0),
                        (sc[:, 1, qsl], 2),
                        (sc[:, 0, qsl], 4),
                        (mg256[0:1, qsl], 6),
                        (cs256[0:1, qsl], 7),
                        (mag[:, 1, qsl], 1),
                        (sc[:, 3, qsl], 3),
                        (sc[:, 2, qsl], 5),
                    ]
                    ps_y = psy.tile([P, NF], F32, tag="y", name="psy")
                    for i, (lhsT, kt) in enumerate(klist):
                        nc.tensor.matmul(
                            ps_y[:],
                            lhsT,
                            ap_sb[0:lhsT.partition_size(), kt, :],
                            start=(i == 0),
                            stop=(i == len(klist) - 1),
                        )
                    # e = exp(silu(y)) via tanh; accum_out gives the row sum
                    th = wpool.tile([P, NF], F32, tag="th", name="th", bufs=3)
                    nc.scalar.activation(th[:], ps_y[:], func=AF.Tanh, scale=0.5)
                    ysw = wpool.tile([P, NF], F32, tag="ysw", name="ysw", bufs=3)
                    nc.vector.scalar_tensor_tensor(
                        out=ysw[:], in0=th[:], scalar=1.0, in1=ps_y[:],
                        op0=mybir.AluOpType.add, op1=mybir.AluOpType.mult,
                    )
                    e = wpool.tile([P, NF], BF16, tag="e", name="e", bufs=3)
                    s = spool.tile([P, 1], F32, tag=f"s_{c}_{q}", name=f"s_{c}_{q}")
                    nc.scalar.activation(e[:], ysw[:], func=AF.Exp, scale=0.5,
                                         accum_out=s[:])
                    srec = spool.tile([P, 1], F32, tag=f"sr_{c}_{q}",
                                      name=f"sr_{c}_{q}")
                    recip_fast(srec[:], s[:])
                    w = wpool.tile([P, NF], BF16, tag="w", name="w", bufs=8)
                    with nc.allow_low_precision(reason="softmax normalize bf16"):
                        nc.vector.tensor_scalar_mul(w[:], e[:], srec[:])
                    wq.append(w)
                wqs[c] = wq

            # ===== stage 4: per chunk: w-transpose -> u -> inverse -> DMA
            # (chunk 1's transposes wait on its softmax while chunk 0's
            # inverse keeps the PE busy)
            for c in range(NCH):
                wT = pswt.tile([P, 2, CH], BF16, tag="wt", name="wt")
                wnT = pswn.tile([1, CH], BF16, tag="wnt", name="wnt")
                for q in range(NQ):
                    qsl = slice(q * P, (q + 1) * P)
                    w = wqs[c][q]
                    nc.tensor.transpose(wT[:, 0, qsl], w[:, 0:P], eye_sb[:])
                    nc.tensor.transpose(wT[:, 1, qsl], w[:, P:2 * P], eye_sb[:])
                    nc.tensor.transpose(wnT[0:1, qsl], w[:, 2 * P:NF], eye_sb[:])

                Ua = fpool.tile([P, 2, CH], BF16, tag=f"ua_{c}", name=f"ua_{c}")
                Ub = fpool.tile([P, 2, CH], BF16, tag=f"ub_{c}", name=f"ub_{c}")
                with nc.allow_low_precision(reason="u bf16"):
                    nc.vector.tensor_mul(
                        Ua[:], Tc[c][:, 0:2, :],
                        wT[:, 0, None, :].broadcast_to([P, 2, CH]),
                    )
                    # nyquist u row: re256 * w256 rides the im[0] slot
                    nc.vector.tensor_mul(Ua[0:1, 1, :], Tc[c][0:1, 1, :], wnT[:])
                    nc.vector.tensor_mul(
                        Ub[:], Tc[c][:, 2:4, :],
                        wT[:, 1, None, :].broadcast_to([P, 2, CH]),
                    )

                U = [Ua[:, 0, :], Ub[:, 0, :], Ua[:, 1, :], Ub[:, 1, :]]
                korder = [0, 2, 1, 3]
                for mt in range(4):
                    ps = psmm.tile([P, CH], F32, tag="mm", name="psz")
                    for j, k in enumerate(korder):
                        nc.tensor.matmul(
                            ps[:],
                            wi_sb[:, k, mt * P:(mt + 1) * P],
                            U[k],
                            start=(j == 0),
                            stop=(j == 3),
                        )
                    if mt in (0, 2):
                        nc.scalar.activation(zout[mt][:, c, :], ps[:], func=AF.Copy)
                    else:
                        nc.vector.tensor_copy(zout[mt][:, c, :], ps[:])
                    eng = nc.gpsimd
                    eng.dma_start(
                        out_ext[mt][:, c * (CH // F):(c + 1) * (CH // F), :],
                        zout[mt][:, c, :].rearrange("p (b f) -> p b f", f=F),
                    )

    nc.compile()
    return nc


_CACHE = {}


def _bf16():
    import ml_dtypes
    return ml_dtypes.bfloat16


def _pack_in_maps(inputs):
    Wf, Ap, Wi, eye = _build_constants(
        inputs["W_proj"], inputs["b_proj"], inputs["W_gate"], inputs["b_gate"]
    )
    x = np.ascontiguousarray(np.asarray(inputs["x"], np.float32))
    return [
        {
            # (BL,H,F) -> (4,P,BL,F): h-tile-major, partition-contiguous
            "x": np.ascontiguousarray(
                x[c * BL:(c + 1) * BL].transpose(1, 0, 2).reshape(4, P, BL, F)
            ).astype(_bf16()),
            "wf": Wf,
            "apk": Ap,
            "wi": Wi,
            "eye": eye,
        }
        for c in range(NCORES)
    ]


def _run(inputs, trace=False):
    from concourse.bass_utils import run_bass_kernel_spmd

    if "graph" not in _CACHE:
        _CACHE["graph"] = _build_graph()
    nc = _CACHE["graph"]
    in_maps = _pack_in_maps(inputs)
    res = run_bass_kernel_spmd(nc, in_maps, core_ids=list(range(NCORES)), trace=trace)
    # (4,P,BL,F) -> (BL,H,F)
    out = np.concatenate(
        [r["out"].transpose(2, 0, 1, 3).reshape(BL, H, F) for r in res.results],
        axis=0,
    )
    return out.astype(np.float32), res


def kernel(**inputs):
    out, _ = _run(inputs, trace=False)
    return out


def _make_exec(nc):
    """Build a jit-cached 8-core executor for a compiled Bacc graph,
    replicating bass2jax.run_bass_via_pjrt's multi-core path but reusable
    across calls (for timing)."""
    import jax
    import numpy as np
    from jax.sharding import Mesh, PartitionSpec
    from jax.experimental.shard_map import shard_map
    from concourse import mybir
    from concourse.bass2jax import _bass_exec_p, install_neuronx_cc_hook

    install_neuronx_cc_hook()
    from concourse.bass2jax import partition_id_tensor

    n_cores = NCORES
    pid_name = nc.partition_id_tensor.name if nc.partition_id_tensor else None
    in_names, out_names, out_avals, zero_outs = [], [], [], []
    for alloc in nc.m.functions[0].allocations:
        if not isinstance(alloc, mybir.MemoryLocationSet):
            continue
        name = alloc.memorylocations[0].name
        if alloc.kind == "ExternalInput":
            if name != pid_name:
                in_names.append(name)
        elif alloc.kind == "ExternalOutput":
            out_names.append(name)
            shape = tuple(alloc.tensor_shape)
            dtype = mybir.dt.np(alloc.dtype)
            out_avals.append(jax.core.ShapedArray(shape, dtype))
            zero_outs.append(np.zeros(shape, dtype))
    n_params = len(in_names)
    all_names = in_names + out_names
    if pid_name is not None:
        all_names = all_names + [pid_name]

    def _body(*args):
        operands = list(args)
        if pid_name is not None:
            operands.append(partition_id_tensor())
        outs = _bass_exec_p.bind(
            *operands,
            out_avals=tuple(out_avals),
            in_names=tuple(all_names),
            out_names=tuple(out_names),
            lowering_input_output_aliases=(),
            sim_require_finite=True,
            sim_require_nnan=True,
            nc=nc,
        )
        return tuple(outs)

    devices = jax.devices()[:n_cores]
    mesh = Mesh(np.asarray(devices), ("core",))
    n_all = n_params + len(out_names)
    fn = jax.jit(
        shard_map(
            _body,
            mesh=mesh,
            in_specs=(PartitionSpec("core"),) * n_all,
            out_specs=(PartitionSpec("core"),) * len(out_names),
            check_rep=False,
        ),
        keep_unused=True,
    )

    def pack(in_maps):
        concat = [
            np.concatenate([np.asarray(in_maps[c][k]) for c in range(n_cores)], axis=0)
            for k in in_names
        ]
        concat += [
            np.zeros((n_cores * z.shape[0], *z.shape[1:]), z.dtype) for z in zero_outs
        ]
        return [jax.device_put(a) for a in concat]

    return fn, pack, out_names, out_avals
